# revision 37
# baseline (speedup 1.0000x reference)
"""Causal multi-head self-attention with RoPE on 8 Trainium2 NeuronCores.

Problem: b=4, s=2048, d_model=1024, 16 heads, dk=64, causal, RoPE(theta=1e4).

Sharding: 8 cores = (batch, head-half). Core c handles batch c//2 and heads
(c%2)*8 .. +8: QKV projections, causal attention, partial output projection;
the host sums the two partials per batch.

Structure (v4, ~316us vs 385us baseline):
- Anti-diagonal wavefront over (head-pair group, q-chunk) so the exp load on
  the scalar engine (the per-block pacer, ~1us per 128-key block) is uniform
  across the kernel instead of piling up in the late, feeder-poor chunks.
- A projection "feeder": Q/K/V/Wo matmuls sliced into ~1us closures, pulled
  into the PE stream between attention blocks by a per-block ns budget, with
  forced drains at each dependency edge. Single-accumulator projection runs
  stream at ~215ns/matmul with LDWEIGHTS fully hidden.
- Scores for block kt are emitted before PV of block kt-1 (one-block softmax
  lookahead, PSUM scores pool depth 2).
- Softmax denominators ride ones-column matmuls packed 4-wide across kt pairs
  (PE column strips 0/32/64/96), one w-column pass per two blocks.
- Den reciprocal chain: PSUM rows -> 8-wide repack (DVE reciprocal is 8
  cyc/el) -> one flatten DMA -> gpsimd partition_broadcast (library pre-warmed
  at startup; the first use otherwise stalls ~13us on LOAD_LIB) -> two
  half-tile normalize multiplies. No DRAM bounce.
- Inputs land via one multi-dim DMA per tensor-chunk: each dma_start costs
  ~0.6us of dispatch on its issuing engine, so descriptor count >> dispatch
  count. Dispatches are spread across sync/gpsimd queues; the exp activation
  table is pre-warmed during the input DMA window.
- Output projection: spread into the next chunk attention window via the
  feeder; the final chunk runs in 4-unit waves on 4 PSUM accumulators with
  two chunk-2 token tiles reserved as tail filler, evacuated on the (idle)
  scalar engine.
"""
import sys
import numpy as np

for _p in ('/root/.axon_site/_ro/trn_rl_repo', '/opt/trn_rl_repo'):
    if _p not in sys.path:
        sys.path.append(_p)

import concourse.bass as bass
import concourse.tile as tile
from concourse import bacc, mybir
from concourse.bass_utils import run_bass_kernel_spmd

F32 = mybir.dt.float32
BF16 = mybir.dt.bfloat16
EXP = mybir.ActivationFunctionType.Exp
MUL = mybir.AluOpType.mult

B, S, D = 4, 2048, 1024
NH, DK = 16, 64
NHC = 8            # heads per core
HD = NHC * DK      # 512
NG = 4             # head-pairs per core
NC = 512           # q-chunk
N_CHUNKS = S // NC
N_KT = S // 128
KSUB = D // 128
THETA = 10000.0

_CACHED = {}


def _build():
    nc = bacc.Bacc('TRN2', target_bir_lowering=False, debug=False, num_devices=8)
    xT = nc.dram_tensor('xT', [D, S], BF16, kind='ExternalInput').ap()
    wqT = nc.dram_tensor('wqT', [D, HD], BF16, kind='ExternalInput').ap()
    wkT = nc.dram_tensor('wkT', [D, HD], BF16, kind='ExternalInput').ap()
    wvT = nc.dram_tensor('wvT', [D, HD], BF16, kind='ExternalInput').ap()
    woT = nc.dram_tensor('woT', [HD, D], BF16, kind='ExternalInput').ap()
    cosd = nc.dram_tensor('cosd', [128, S], BF16, kind='ExternalInput').ap()
    sind = nc.dram_tensor('sind', [128, S], BF16, kind='ExternalInput').ap()
    maskd = nc.dram_tensor('maskd', [128, 128], BF16, kind='ExternalInput').ap()
    y = nc.dram_tensor('y', [S, D], BF16, kind='ExternalOutput').ap()
    recip_d = nc.dram_tensor('recip_d', [NG, N_CHUNKS, 2, NC], BF16).ap()

    with tile.TileContext(nc) as tc:
        with tc.tile_pool(name='persist', bufs=1) as persist, \
             tc.tile_pool(name='rope', bufs=3) as ropep, \
             tc.tile_pool(name='pp', bufs=8) as pp, \
             tc.tile_pool(name='dent', bufs=2) as dent, \
             tc.tile_pool(name='rct', bufs=3) as rct, \
             tc.tile_pool(name='ytp', bufs=4) as ytp, \
             tc.tile_pool(name='scps', bufs=2, space='PSUM') as scps, \
             tc.tile_pool(name='pvps', bufs=1, space='PSUM') as pvps, \
             tc.tile_pool(name='denps', bufs=1, space='PSUM') as denps, \
             tc.tile_pool(name='projps', bufs=2, space='PSUM') as projps:

            # ---------- persistent tiles; one multi-dim DMA per chunk ------
            wqk = {}

            def load_w(g, which):
                w_ap = wqT if which == 0 else wkT
                nm = 'wq' if which == 0 else 'wk'
                wt = persist.tile([128, KSUB, 128], BF16, tag=f'{nm}{g}',
                                  name=f'{nm}{g}')
                src = bass.AP(tensor=w_ap.tensor, offset=128 * g,
                              ap=[[HD, 128], [128 * HD, KSUB], [1, 128]])
                nc.sync.dma_start(wt[:], src)
                wqk[(g, which)] = wt

            x_sb = persist.tile([128, KSUB, S], BF16, tag='x_sb')
            cos_sb = persist.tile([128, S], BF16, tag='cos_sb')
            sin_sb = persist.tile([128, S], BF16, tag='sin_sb')
            tri_sb = persist.tile([128, 128], BF16, tag='tri_sb')
            ones_sb = persist.tile([128, 1], BF16, tag='ones_sb')
            v_sb = persist.tile([128, N_KT, NHC, DK], BF16, tag='v_sb')
            lhs_sb = persist.tile([128, NG, S], BF16, tag='lhs_sb')
            wo_sb = persist.tile([128, NG, D], BF16, tag='wo_sb')
            wv_sb = persist.tile([128, KSUB, HD], BF16, tag='wv_sb')

            def dma_x(c, split=1):
                for h in range(split):
                    s0, s1 = h * KSUB // split, (h + 1) * KSUB // split
                    src = bass.AP(tensor=xT.tensor,
                                  offset=NC * c + s0 * 128 * S,
                                  ap=[[S, 128], [128 * S, s1 - s0], [1, NC]])
                    nc.sync.dma_start(x_sb[:, s0:s1, NC * c:NC * (c + 1)],
                                      src)

            # startup-critical first
            load_w(0, 0)
            load_w(0, 1)
            dma_x(0, split=2)
            nc.sync.dma_start(
                wv_sb[:], bass.AP(tensor=wvT.tensor, offset=0,
                                  ap=[[HD, 128], [128 * HD, KSUB], [1, HD]]))
            nc.sync.dma_start(cos_sb[:, 0:2 * NC], cosd[:, 0:2 * NC])
            nc.sync.dma_start(sin_sb[:, 0:2 * NC], sind[:, 0:2 * NC])
            nc.sync.dma_start(tri_sb[:], maskd)
            dma_x(1)
            nc.vector.memset(ones_sb[:], 1.0)
            # pre-warm the exp activation table during the DMA window
            wrm = dent.tile([128, 8], BF16, tag='warm')
            wrm2 = dent.tile([128, 8], BF16, tag='warm2')
            nc.vector.memset(wrm[:], 0.0)
            nc.scalar.activation(out=wrm2[:], in_=wrm[:], func=EXP, scale=1.0)
            wrm3 = dent.tile([128, 8], BF16, tag='warm3')
            nc.gpsimd.partition_broadcast(wrm3[:], wrm[0:1, :])
            load_w(1, 0)
            load_w(1, 1)
            dma_x(2)
            dma_x(3)
            nc.sync.dma_start(cos_sb[:, 2 * NC:S], cosd[:, 2 * NC:S])
            nc.sync.dma_start(sin_sb[:, 2 * NC:S], sind[:, 2 * NC:S])
            load_w(2, 0)
            load_w(2, 1)
            load_w(3, 0)
            load_w(3, 1)
            nc.sync.dma_start(
                wo_sb[:], bass.AP(tensor=woT.tensor, offset=0,
                                  ap=[[D, 128], [128 * D, NG], [1, D]]))

            qk_tiles = {}
            for g in range(NG):
                qk_tiles[g] = [
                    persist.tile([128, S], BF16, tag=f'q{g}', name=f'q{g}'),
                    persist.tile([128, S], BF16, tag=f'k{g}', name=f'k{g}')]

            # ---------- feeder: projection work sliced into ~1us closures ----
            feeder = []          # list of [tag_or_None, cost_ns, closure]
            done_tags = set()
            state = {'ns': 0.0,
                     'kts': sum(4 * (c + 1) for c in range(N_CHUNKS)) * NG}

            def push(closures):
                for t, cost, fn in closures:
                    feeder.append((t, cost, fn))
                    state['ns'] += cost

            def run_next():
                t, cost, fn = feeder.pop(0)
                fn()
                state['ns'] -= cost
                if t is not None:
                    if isinstance(t, list):
                        done_tags.update(t)
                    else:
                        done_tags.add(t)

            def drain(tag):
                while tag not in done_tags:
                    assert feeder, f'feeder empty before {tag}'
                    run_next()

            def pull(budget):
                while feeder and budget > 0:
                    budget -= feeder[0][1]
                    run_next()

            def prep_item(g, which, chunks, evac):
                """Q/K projection + RoPE for the given chunks of group g."""
                wt = wqk[(g, which)]
                dst = qk_tiles[g][which]
                c0, c1 = chunks[0], chunks[-1] + 1
                cs2 = slice(NC * c0, NC * c1)
                box = {}

                def mk_mm_a(c):
                    def fn():
                        box['ps'] = projps.tile([128, NC], F32, tag='proj',
                                                name=f'pqk{g}_{which}_{c}')
                        for s in range(4):
                            nc.tensor.matmul(
                                box['ps'][:], wt[:, s, :],
                                x_sb[:, s, NC * c:NC * (c + 1)],
                                start=(s == 0), stop=False)
                    return fn

                def mk_mm_b(c):
                    def fn():
                        for s in range(4, KSUB):
                            nc.tensor.matmul(
                                box['ps'][:], wt[:, s, :],
                                x_sb[:, s, NC * c:NC * (c + 1)],
                                start=False, stop=(s == KSUB - 1))
                        dcol = slice(NC * c, NC * (c + 1))
                        if evac == 'scalar':
                            nc.scalar.copy(dst[:, dcol], box['ps'][:])
                        else:
                            nc.vector.tensor_copy(dst[:, dcol], box['ps'][:])
                    return fn

                def rope_fn():
                    wid = NC * (c1 - c0)
                    sw = ropep.tile([128, 2 * NC], BF16, tag='sw')
                    for blk in range(4):
                        sp = (blk // 2) * 64 + (1 - blk % 2) * 32
                        nc.sync.dma_start(sw[32 * blk:32 * (blk + 1), 0:wid],
                                          dst[sp:sp + 32, cs2])
                    t1 = ropep.tile([128, 2 * NC], BF16, tag='t1')
                    nc.vector.tensor_mul(t1[:, 0:wid], dst[:, cs2],
                                         cos_sb[:, cs2])
                    t2 = ropep.tile([128, 2 * NC], BF16, tag='t2')
                    nc.vector.tensor_tensor(t2[:, 0:wid], sw[:, 0:wid],
                                            sin_sb[:, cs2], MUL)
                    nc.vector.tensor_add(dst[:, cs2], t1[:, 0:wid],
                                         t2[:, 0:wid])

                out = []
                for c in chunks:
                    out.append((None, 1100, mk_mm_a(c)))
                    out.append((None, 1400, mk_mm_b(c)))
                out.append(([('pc', g, which, c) for c in chunks], 900,
                            rope_fn))
                return out

            def v_item(t, evac):
                """V projection for token tile t (all 8 heads)."""
                box = {}

                def a():
                    box['ps'] = projps.tile([128, NC], F32, tag='proj',
                                            name=f'pv{t}')
                    for s in range(4):
                        nc.tensor.matmul(
                            box['ps'][:], x_sb[:, s, 128 * t:128 * (t + 1)],
                            wv_sb[:, s, :], start=(s == 0), stop=False)

                def b():
                    for s in range(4, KSUB):
                        nc.tensor.matmul(
                            box['ps'][:], x_sb[:, s, 128 * t:128 * (t + 1)],
                            wv_sb[:, s, :], start=False, stop=(s == KSUB - 1))
                    r = box['ps'].rearrange('p (h m) -> p h m', h=NHC)
                    if evac == 'scalar':
                        nc.scalar.copy(v_sb[:, t], r)
                    else:
                        nc.vector.tensor_copy(v_sb[:, t], r)

                return [(None, 1100, a), (('v', t), 1400, b)]

            def wo_item(t):
                """Output projection for token tile t (halves sequential)."""
                ts_ = slice(128 * t, 128 * (t + 1))
                box = {}

                def mk_half_a(half):
                    def fn():
                        box[half] = projps.tile([128, NC], F32, tag='proj',
                                                name=f'pwo{t}_{half}')
                        for g in range(NG):
                            nc.tensor.matmul(
                                box[half][:], lhs_sb[:, g, ts_],
                                wo_sb[:, g, NC * half:NC * (half + 1)],
                                start=(g == 0), stop=(g == NG - 1))
                    return fn

                def mk_half_b(half):
                    def fn():
                        if 'yt' not in box:
                            box['yt'] = ytp.tile([128, D], BF16, tag='yt',
                                                 name=f'yt{t}')
                        nc.vector.tensor_copy(
                            box['yt'][:, NC * half:NC * (half + 1)],
                            box[half][:])
                        if half == 1:
                            nc.gpsimd.dma_start(y[ts_, :], box['yt'][:])
                    return fn

                return [(None, 1100, mk_half_a(0)), (None, 350, mk_half_b(0)),
                        (None, 1100, mk_half_a(1)),
                        (('wo', t), 350, mk_half_b(1))]

            # feeder fill order follows wavefront consumption order
            push(prep_item(0, 0, [0], 'scalar'))
            push(prep_item(0, 1, [0], 'scalar'))
            for t in range(4):
                push(v_item(t, 'scalar'))
            push(prep_item(0, 0, [1], 'scalar'))
            push(prep_item(0, 1, [1], 'scalar'))
            push(prep_item(1, 0, [0, 1], 'vector'))
            push(prep_item(1, 1, [0, 1], 'vector'))
            for t in range(4, 8):
                push(v_item(t, 'scalar'))
            push(prep_item(2, 0, [0, 1], 'vector'))
            push(prep_item(2, 1, [0, 1], 'vector'))
            push(prep_item(0, 0, [2, 3], 'vector'))
            push(prep_item(0, 1, [2, 3], 'vector'))
            for t in range(8, 12):
                push(v_item(t, 'scalar'))
            push(prep_item(3, 0, [0, 1], 'vector'))
            push(prep_item(3, 1, [0, 1], 'vector'))
            push(prep_item(1, 0, [2, 3], 'vector'))
            push(prep_item(1, 1, [2, 3], 'vector'))
            for t in range(12, 16):
                push(v_item(t, 'scalar'))
            push(prep_item(2, 0, [2, 3], 'vector'))
            push(prep_item(2, 1, [2, 3], 'vector'))
            push(prep_item(3, 0, [2, 3], 'vector'))
            push(prep_item(3, 1, [2, 3], 'vector'))

            # ---------- attention ----------
            def emit_att_chunk(g, c, drain_all=False):
                qt, kt_ = qk_tiles[g]
                hA, hB = 2 * g, 2 * g + 1
                cs = slice(NC * c, NC * (c + 1))
                pv = pvps.tile([128, NC], F32, tag='pv')
                den = denps.tile([128, NC], F32, tag='den')
                n_kt = 4 * (c + 1)
                p_tiles = {}

                def emit_sc(i):
                    drain(('v', i))
                    j = i - 4 * c
                    v0 = max(j, 0) * 128
                    w = NC - v0
                    qs = slice(NC * c + v0, NC * (c + 1))
                    ks = slice(128 * i, 128 * (i + 1))
                    sc = scps.tile([128, 2 * NC], F32, tag='sc')
                    nc.tensor.matmul(sc[:, v0:NC], kt_[0:64, ks],
                                     qt[0:64, qs], start=True, stop=True)
                    nc.tensor.matmul(sc[:, NC + v0:2 * NC], kt_[64:128, ks],
                                     qt[64:128, qs], start=True, stop=True)
                    p = pp.tile([128, 2 * NC], BF16, tag='p')
                    p_tiles[i] = p
                    sc_seg = bass.AP(tensor=sc.tensor, offset=sc.offset + v0,
                                     ap=[list(sc.ap[0]), [NC, 2], [1, w]])
                    p_seg = bass.AP(tensor=p.tensor, offset=p.offset + v0,
                                    ap=[list(p.ap[0]), [NC, 2], [1, w]])
                    nc.scalar.activation(out=p_seg, in_=sc_seg, func=EXP,
                                         scale=1.0 / np.sqrt(DK))
                    if j >= 0:       # triangle on the diag sub-block
                        dseg = bass.AP(
                            tensor=p.tensor, offset=p.offset + v0,
                            ap=[list(p.ap[0]), [NC, 2], [1, 128]])
                        nc.gpsimd.tensor_tensor(
                            dseg, dseg,
                            tri_sb[:, None, :].to_broadcast([128, 2, 128]),
                            MUL)

                def emit_pv(i):
                    v0 = max(i - 4 * c, 0) * 128
                    sA = slice(v0, NC)
                    sB = slice(NC + v0, 2 * NC)
                    st_, sp_ = (i == 0), (i == n_kt - 1)
                    p = p_tiles[i]
                    nc.tensor.matmul(pv[0:64, sA], v_sb[:, i, hA, :],
                                     p[:, sA], start=st_, stop=sp_)
                    nc.tensor.matmul(pv[64:128, sA], v_sb[:, i, hB, :],
                                     p[:, sB], start=st_, stop=sp_)

                def emit_den2(i):
                    # c==0: later blocks are width-trimmed, so keep a single
                    # accumulator pair (rows 0/32) that block 0 fully covers
                    v0 = max(i - 4 * c, 0) * 128
                    sA = slice(v0, NC)
                    sB = slice(NC + v0, 2 * NC)
                    st_, sp_ = (i == 0), (i == n_kt - 1)
                    p = p_tiles.pop(i)
                    nc.tensor.matmul(den[0:1, sA], ones_sb[:], p[:, sA],
                                     start=st_, stop=sp_, tile_position=(0, 0))
                    nc.tensor.matmul(den[32:33, sA], ones_sb[:], p[:, sB],
                                     start=st_, stop=sp_, tile_position=(0, 32))

                def emit_den4(m):
                    # kt pair (2m, 2m+1): 4 ones-matmuls packed across PE
                    # column strips run concurrently -> one w-col pass per pair
                    st_, sp_ = (m == 0), (m == n_kt // 2 - 1)
                    for idx, i in enumerate((2 * m, 2 * m + 1)):
                        v0 = max(i - 4 * c, 0) * 128
                        sA = slice(v0, NC)
                        sB = slice(NC + v0, 2 * NC)
                        p = p_tiles.pop(i)
                        ro = 64 * idx
                        nc.tensor.matmul(den[ro:ro + 1, sA], ones_sb[:],
                                         p[:, sA], start=st_, stop=sp_,
                                         tile_position=(0, ro))
                        nc.tensor.matmul(den[ro + 32:ro + 33, sA], ones_sb[:],
                                         p[:, sB], start=st_, stop=sp_,
                                         tile_position=(0, ro + 32))

                for i in range(n_kt):
                    emit_sc(i)
                    # den work lags 2 extra blocks: its p inputs are long
                    # ready, so it streams stall-free between sc and pv
                    if c == 0:
                        if i >= 2:
                            emit_den2(i - 2)
                    elif i >= 4 and i % 2 == 0:
                        emit_den4((i - 4) // 2)
                    if drain_all and i == n_kt // 2:
                        while feeder:
                            run_next()
                    pull(1.25 * state['ns'] / max(state['kts'], 1))
                    state['kts'] -= 1
                    if i >= 1:
                        emit_pv(i - 1)
                emit_pv(n_kt - 1)
                if c == 0:
                    emit_den2(n_kt - 2)
                    emit_den2(n_kt - 1)
                else:
                    emit_den4(n_kt // 2 - 2)
                    emit_den4(n_kt // 2 - 1)

                # evac pv early (frees the single pv bank), normalize later
                nc.vector.tensor_copy(lhs_sb[:, g, cs], pv[:])

                stage = dent.tile([128, NC], F32, tag='dstage')
                hi = 33 if c == 0 else 97
                nc.vector.tensor_copy(stage[0:hi, :], den[0:hi, :])
                packed = dent.tile([128, 8], F32, tag='dpA')
                nc.sync.dma_start(packed[0:64, :], stage[0:1, :])
                nc.sync.dma_start(packed[64:128, :], stage[32:33, :])
                if c > 0:
                    packed2 = dent.tile([128, 8], F32, tag='dpB')
                    nc.gpsimd.dma_start(packed2[0:64, :], stage[64:65, :])
                    nc.gpsimd.dma_start(packed2[64:128, :], stage[96:97, :])
                    nc.vector.tensor_add(packed[:], packed[:], packed2[:])
                packr = dent.tile([128, 8], BF16, tag='dpackr')
                with nc.allow_low_precision(reason='softmax 1/sum in bf16'):
                    nc.vector.reciprocal(packr[:], packed[:])
                flat = dent.tile([128, 2 * NC], BF16, tag='dflat')
                nc.gpsimd.dma_start(flat[0:1, :], packr[:])
                rcf = rct.tile([128, 2 * NC], BF16, tag='rcf')
                nc.gpsimd.partition_broadcast(rcf[:], flat[0:1, :])
                nc.vector.tensor_tensor(lhs_sb[0:64, g, cs],
                                        lhs_sb[0:64, g, cs],
                                        rcf[0:64, 0:NC], MUL)
                nc.vector.tensor_tensor(lhs_sb[64:128, g, cs],
                                        lhs_sb[64:128, g, cs],
                                        rcf[64:128, NC:2 * NC], MUL)

            # ------------- anti-diagonal wavefront -------------
            done_cnt = [0] * N_CHUNKS
            last_diag = N_CHUNKS + NG - 2
            for k in range(N_CHUNKS + NG - 1):
                for g in range(max(0, k - N_CHUNKS + 1), min(NG - 1, k) + 1):
                    c = k - g
                    drain(('pc', g, 0, c))
                    for cc in range(c + 1):
                        drain(('pc', g, 1, cc))
                    emit_att_chunk(g, c, drain_all=(k == last_diag))
                    done_cnt[c] += 1
                    if done_cnt[c] == NG:
                        if c < N_CHUNKS - 1:
                            hi = 4 * (c + 1) - (2 if c == N_CHUNKS - 2 else 0)
                            for t in range(4 * c, hi):
                                push(wo_item(t))
                        else:
                            while feeder:
                                run_next()
                            # t10/t11 (chunk 2, already normalized) lead the
                            # final waves as chain-latency filler
                            units = [(t, half)
                                     for t in range(4 * c - 2, 4 * (c + 1))
                                     for half in range(2)]
                            yts = {}
                            pools = [projps, projps, pvps, denps]
                            tags = ['proj', 'proj', 'pv', 'den']
                            ydisp = [nc.sync, nc.gpsimd]
                            for wi, w0 in enumerate(range(0, len(units), 4)):
                                wave = units[w0:w0 + 4]
                                accs = {}
                                for ui, (t, half) in enumerate(wave):
                                    accs[(t, half)] = pools[ui].tile(
                                        [128, NC], F32, tag=tags[ui],
                                        name=f'pwof{t}_{half}')
                                for gg in range(NG):
                                    for (t, half) in wave:
                                        ts_ = slice(128 * t, 128 * (t + 1))
                                        nc.tensor.matmul(
                                            accs[(t, half)][:],
                                            lhs_sb[:, gg, ts_],
                                            wo_sb[:, gg,
                                                  NC * half:NC * (half + 1)],
                                            start=(gg == 0),
                                            stop=(gg == NG - 1))
                                for (t, half) in wave:
                                    ts_ = slice(128 * t, 128 * (t + 1))
                                    if t not in yts:
                                        yts[t] = ytp.tile([128, D], BF16,
                                                          tag='yt',
                                                          name=f'ytf{t}')
                                    nc.scalar.copy(
                                        yts[t][:, NC * half:NC * (half + 1)],
                                        accs[(t, half)][:])
                                    if half == 1:
                                        ydisp[t % 2].dma_start(y[ts_, :],
                                                               yts[t][:])
    nc.compile()
    return nc


def _host_inputs(x, Wq, Wk, Wv, Wo, token_positions):
    """Per-core input maps (host-side sharding / layout / dtype prep only)."""
    import ml_dtypes
    bf16 = ml_dtypes.bfloat16
    perm = np.empty(DK, np.int64)
    perm[0:32] = np.arange(0, DK, 2)
    perm[32:64] = np.arange(1, DK, 2)

    inv_freq = 1.0 / (THETA ** (np.arange(0, DK, 2, dtype=np.float64) / DK))  # [32]
    ang = token_positions.astype(np.float64)[None, :] * inv_freq[:, None]     # [32, S]
    cos32 = np.cos(ang).astype(np.float32)
    sin32 = np.sin(ang).astype(np.float32)
    cos128 = np.tile(cos32, (4, 1)).astype(bf16)
    sin128 = np.concatenate([-sin32, sin32, -sin32, sin32], axis=0).astype(bf16)

    tri = (np.arange(128)[None, :] >= np.arange(128)[:, None]).astype(bf16)

    in_maps = []
    for core in range(8):
        b = core // 2
        h0 = (core % 2) * NHC
        cols = slice(h0 * DK, (h0 + NHC) * DK)
        wq_s = Wq[cols, :].reshape(NHC, DK, D)[:, perm, :].reshape(HD, D)
        wk_s = Wk[cols, :].reshape(NHC, DK, D)[:, perm, :].reshape(HD, D)
        in_maps.append({
            'xT': np.ascontiguousarray(x[b].T.astype(bf16)),
            'wqT': np.ascontiguousarray(wq_s.T.astype(bf16)),
            'wkT': np.ascontiguousarray(wk_s.T.astype(bf16)),
            'wvT': np.ascontiguousarray(Wv[cols, :].T.astype(bf16)),
            'woT': np.ascontiguousarray(Wo[:, cols].T.astype(bf16)),
            'cosd': cos128, 'sind': sin128, 'maskd': tri,
        })
    return in_maps


def kernel(x, Wq, Wk, Wv, Wo, token_positions, _results_hook=None):
    if 'nc' not in _CACHED:
        _CACHED['nc'] = _build()
    nc = _CACHED['nc']
    in_maps = _host_inputs(np.asarray(x), np.asarray(Wq), np.asarray(Wk),
                           np.asarray(Wv), np.asarray(Wo),
                           np.asarray(token_positions))
    res = run_bass_kernel_spmd(nc, in_maps, list(range(8)),
                               **(_results_hook or {}))
    if _results_hook is not None:
        _CACHED['last'] = res
    out = np.empty((B, S, D), np.float32)
    for b in range(B):
        out[b] = (res.results[2 * b]['y'].astype(np.float32)
                  + res.results[2 * b + 1]['y'].astype(np.float32))
    return out


# revision 38
# speedup vs baseline: 1.5384x; 1.5384x over previous
"""Causal multi-head self-attention with RoPE on 8 Trainium2 NeuronCores.

Problem: b=4, s=2048, d_model=1024, 16 heads, dk=64, causal, RoPE(theta=1e4).

Sharding: 8 cores = (batch, head-half). Core c handles batch c//2 and heads
(c%2)*8 .. +8: QKV projections, causal attention, partial output projection;
the host sums the two partials per batch.

Structure (v4, ~316us vs 385us baseline):
- Anti-diagonal wavefront over (head-pair group, q-chunk) so the exp load on
  the scalar engine (the per-block pacer, ~1us per 128-key block) is uniform
  across the kernel instead of piling up in the late, feeder-poor chunks.
- A projection "feeder": Q/K/V/Wo matmuls sliced into ~1us closures, pulled
  into the PE stream between attention blocks by a per-block ns budget, with
  forced drains at each dependency edge. Single-accumulator projection runs
  stream at ~215ns/matmul with LDWEIGHTS fully hidden.
- Scores for block kt are emitted before PV of block kt-1 (one-block softmax
  lookahead, PSUM scores pool depth 2).
- Softmax denominators ride ones-column matmuls packed 4-wide across kt pairs
  (PE column strips 0/32/64/96), one w-column pass per two blocks.
- Den reciprocal chain: PSUM rows -> 8-wide repack (DVE reciprocal is 8
  cyc/el) -> one flatten DMA -> gpsimd partition_broadcast (library pre-warmed
  at startup; the first use otherwise stalls ~13us on LOAD_LIB) -> two
  half-tile normalize multiplies. No DRAM bounce.
- Inputs land via one multi-dim DMA per tensor-chunk: each dma_start costs
  ~0.6us of dispatch on its issuing engine, so descriptor count >> dispatch
  count. Dispatches are spread across sync/gpsimd queues; the exp activation
  table is pre-warmed during the input DMA window.
- Output projection: spread into the next chunk attention window via the
  feeder; the final chunk runs in 4-unit waves on 4 PSUM accumulators with
  two chunk-2 token tiles reserved as tail filler, evacuated on the (idle)
  scalar engine.
"""
import sys
import numpy as np

for _p in ('/root/.axon_site/_ro/trn_rl_repo', '/opt/trn_rl_repo'):
    if _p not in sys.path:
        sys.path.append(_p)

import concourse.bass as bass
import concourse.tile as tile
from concourse import bacc, mybir
from concourse.bass_utils import run_bass_kernel_spmd

F32 = mybir.dt.float32
BF16 = mybir.dt.bfloat16
EXP = mybir.ActivationFunctionType.Exp
MUL = mybir.AluOpType.mult

B, S, D = 4, 2048, 1024
NH, DK = 16, 64
NHC = 8            # heads per core
HD = NHC * DK      # 512
NG = 4             # head-pairs per core
NC = 512           # q-chunk
N_CHUNKS = S // NC
N_KT = S // 128
KSUB = D // 128
THETA = 10000.0

_CACHED = {}


def _build():
    nc = bacc.Bacc('TRN2', target_bir_lowering=False, debug=False, num_devices=8)
    xT = nc.dram_tensor('xT', [D, S], BF16, kind='ExternalInput').ap()
    wqT = nc.dram_tensor('wqT', [D, HD], BF16, kind='ExternalInput').ap()
    wkT = nc.dram_tensor('wkT', [D, HD], BF16, kind='ExternalInput').ap()
    wvT = nc.dram_tensor('wvT', [D, HD], BF16, kind='ExternalInput').ap()
    woT = nc.dram_tensor('woT', [HD, D], BF16, kind='ExternalInput').ap()
    cosd = nc.dram_tensor('cosd', [128, S], BF16, kind='ExternalInput').ap()
    sind = nc.dram_tensor('sind', [128, S], BF16, kind='ExternalInput').ap()
    maskd = nc.dram_tensor('maskd', [128, 128], BF16, kind='ExternalInput').ap()
    y = nc.dram_tensor('y', [S, D], BF16, kind='ExternalOutput').ap()
    recip_d = nc.dram_tensor('recip_d', [NG, N_CHUNKS, 2, NC], BF16).ap()

    with tile.TileContext(nc) as tc:
        with tc.tile_pool(name='persist', bufs=1) as persist, \
             tc.tile_pool(name='rope', bufs=3) as ropep, \
             tc.tile_pool(name='pp', bufs=8) as pp, \
             tc.tile_pool(name='dent', bufs=2) as dent, \
             tc.tile_pool(name='rct', bufs=3) as rct, \
             tc.tile_pool(name='ytp', bufs=4) as ytp, \
             tc.tile_pool(name='scps', bufs=2, space='PSUM') as scps, \
             tc.tile_pool(name='pvps', bufs=1, space='PSUM') as pvps, \
             tc.tile_pool(name='denps', bufs=1, space='PSUM') as denps, \
             tc.tile_pool(name='projps', bufs=2, space='PSUM') as projps:

            # ---------- persistent tiles; one multi-dim DMA per chunk ------
            wqk = {}

            def load_w(g, which):
                w_ap = wqT if which == 0 else wkT
                nm = 'wq' if which == 0 else 'wk'
                wt = persist.tile([128, KSUB, 128], BF16, tag=f'{nm}{g}',
                                  name=f'{nm}{g}')
                src = bass.AP(tensor=w_ap.tensor, offset=128 * g,
                              ap=[[HD, 128], [128 * HD, KSUB], [1, 128]])
                nc.sync.dma_start(wt[:], src)
                wqk[(g, which)] = wt

            x_sb = persist.tile([128, KSUB, S], BF16, tag='x_sb')
            cos_sb = persist.tile([128, S], BF16, tag='cos_sb')
            sin_sb = persist.tile([128, S], BF16, tag='sin_sb')
            tri_sb = persist.tile([128, 128], BF16, tag='tri_sb')
            ones_sb = persist.tile([128, 1], BF16, tag='ones_sb')
            v_sb = persist.tile([128, N_KT, NHC, DK], BF16, tag='v_sb')
            lhs_sb = persist.tile([128, NG, S], BF16, tag='lhs_sb')
            wo_sb = persist.tile([128, NG, D], BF16, tag='wo_sb')
            wv_sb = persist.tile([128, KSUB, HD], BF16, tag='wv_sb')

            def dma_x(c, split=1):
                for h in range(split):
                    s0, s1 = h * KSUB // split, (h + 1) * KSUB // split
                    src = bass.AP(tensor=xT.tensor,
                                  offset=NC * c + s0 * 128 * S,
                                  ap=[[S, 128], [128 * S, s1 - s0], [1, NC]])
                    nc.sync.dma_start(x_sb[:, s0:s1, NC * c:NC * (c + 1)],
                                      src)

            # startup-critical first
            load_w(0, 0)
            load_w(0, 1)
            dma_x(0, split=2)
            nc.sync.dma_start(
                wv_sb[:], bass.AP(tensor=wvT.tensor, offset=0,
                                  ap=[[HD, 128], [128 * HD, KSUB], [1, HD]]))
            nc.sync.dma_start(cos_sb[:, 0:2 * NC], cosd[:, 0:2 * NC])
            nc.sync.dma_start(sin_sb[:, 0:2 * NC], sind[:, 0:2 * NC])
            nc.sync.dma_start(tri_sb[:], maskd)
            dma_x(1)
            nc.vector.memset(ones_sb[:], 1.0)
            # pre-warm the exp activation table during the DMA window
            wrm = dent.tile([128, 8], BF16, tag='warm')
            wrm2 = dent.tile([128, 8], BF16, tag='warm2')
            nc.vector.memset(wrm[:], 0.0)
            nc.scalar.activation(out=wrm2[:], in_=wrm[:], func=EXP, scale=1.0)
            wrm3 = dent.tile([128, 8], BF16, tag='warm3')
            nc.gpsimd.partition_broadcast(wrm3[:], wrm[0:1, :])
            load_w(1, 0)
            load_w(1, 1)
            dma_x(2)
            dma_x(3)
            nc.sync.dma_start(cos_sb[:, 2 * NC:S], cosd[:, 2 * NC:S])
            nc.sync.dma_start(sin_sb[:, 2 * NC:S], sind[:, 2 * NC:S])
            load_w(2, 0)
            load_w(2, 1)
            load_w(3, 0)
            load_w(3, 1)
            nc.sync.dma_start(
                wo_sb[:], bass.AP(tensor=woT.tensor, offset=0,
                                  ap=[[D, 128], [128 * D, NG], [1, D]]))

            qk_tiles = {}
            for g in range(NG):
                qk_tiles[g] = [
                    persist.tile([128, S], BF16, tag=f'q{g}', name=f'q{g}'),
                    persist.tile([128, S], BF16, tag=f'k{g}', name=f'k{g}')]

            # ---------- feeder: projection work sliced into ~1us closures ----
            feeder = []          # list of [tag_or_None, cost_ns, closure]
            done_tags = set()
            state = {'ns': 0.0,
                     'kts': sum(4 * (c + 1) for c in range(N_CHUNKS)) * NG}

            def push(closures):
                for t, cost, fn in closures:
                    feeder.append((t, cost, fn))
                    state['ns'] += cost

            def run_next():
                t, cost, fn = feeder.pop(0)
                fn()
                state['ns'] -= cost
                if t is not None:
                    if isinstance(t, list):
                        done_tags.update(t)
                    else:
                        done_tags.add(t)

            def drain(tag):
                while tag not in done_tags:
                    assert feeder, f'feeder empty before {tag}'
                    run_next()

            def pull(budget):
                while feeder and budget > 0:
                    budget -= feeder[0][1]
                    run_next()

            def prep_item(g, which, chunks, evac):
                """Q/K projection + RoPE for the given chunks of group g."""
                wt = wqk[(g, which)]
                dst = qk_tiles[g][which]
                c0, c1 = chunks[0], chunks[-1] + 1
                cs2 = slice(NC * c0, NC * c1)
                box = {}

                def mk_mm_a(c):
                    def fn():
                        box['ps'] = projps.tile([128, NC], F32, tag='proj',
                                                name=f'pqk{g}_{which}_{c}')
                        for s in range(4):
                            nc.tensor.matmul(
                                box['ps'][:], wt[:, s, :],
                                x_sb[:, s, NC * c:NC * (c + 1)],
                                start=(s == 0), stop=False)
                    return fn

                def mk_mm_b(c):
                    def fn():
                        for s in range(4, KSUB):
                            nc.tensor.matmul(
                                box['ps'][:], wt[:, s, :],
                                x_sb[:, s, NC * c:NC * (c + 1)],
                                start=False, stop=(s == KSUB - 1))
                        dcol = slice(NC * c, NC * (c + 1))
                        if evac == 'scalar':
                            nc.scalar.copy(dst[:, dcol], box['ps'][:])
                        else:
                            nc.vector.tensor_copy(dst[:, dcol], box['ps'][:])
                    return fn

                def rope_fn():
                    wid = NC * (c1 - c0)
                    sw = ropep.tile([128, 2 * NC], BF16, tag='sw')
                    for blk in range(4):
                        sp = (blk // 2) * 64 + (1 - blk % 2) * 32
                        nc.sync.dma_start(sw[32 * blk:32 * (blk + 1), 0:wid],
                                          dst[sp:sp + 32, cs2])
                    t1 = ropep.tile([128, 2 * NC], BF16, tag='t1')
                    nc.vector.tensor_mul(t1[:, 0:wid], dst[:, cs2],
                                         cos_sb[:, cs2])
                    t2 = ropep.tile([128, 2 * NC], BF16, tag='t2')
                    nc.vector.tensor_tensor(t2[:, 0:wid], sw[:, 0:wid],
                                            sin_sb[:, cs2], MUL)
                    nc.vector.tensor_add(dst[:, cs2], t1[:, 0:wid],
                                         t2[:, 0:wid])

                out = []
                for c in chunks:
                    out.append((None, 1100, mk_mm_a(c)))
                    out.append((None, 1400, mk_mm_b(c)))
                out.append(([('pc', g, which, c) for c in chunks], 900,
                            rope_fn))
                return out

            def v_item(t, evac):
                """V projection for token tile t (all 8 heads)."""
                box = {}

                def a():
                    box['ps'] = projps.tile([128, NC], F32, tag='proj',
                                            name=f'pv{t}')
                    for s in range(4):
                        nc.tensor.matmul(
                            box['ps'][:], x_sb[:, s, 128 * t:128 * (t + 1)],
                            wv_sb[:, s, :], start=(s == 0), stop=False)

                def b():
                    for s in range(4, KSUB):
                        nc.tensor.matmul(
                            box['ps'][:], x_sb[:, s, 128 * t:128 * (t + 1)],
                            wv_sb[:, s, :], start=False, stop=(s == KSUB - 1))
                    r = box['ps'].rearrange('p (h m) -> p h m', h=NHC)
                    if evac == 'scalar':
                        nc.scalar.copy(v_sb[:, t], r)
                    else:
                        nc.vector.tensor_copy(v_sb[:, t], r)

                return [(None, 1100, a), (('v', t), 1400, b)]

            def wo_item(t):
                """Output projection for token tile t (halves sequential)."""
                ts_ = slice(128 * t, 128 * (t + 1))
                box = {}

                def mk_half_a(half):
                    def fn():
                        box[half] = projps.tile([128, NC], F32, tag='proj',
                                                name=f'pwo{t}_{half}')
                        for g in range(NG):
                            nc.tensor.matmul(
                                box[half][:], lhs_sb[:, g, ts_],
                                wo_sb[:, g, NC * half:NC * (half + 1)],
                                start=(g == 0), stop=(g == NG - 1))
                    return fn

                def mk_half_b(half):
                    def fn():
                        if 'yt' not in box:
                            box['yt'] = ytp.tile([128, D], BF16, tag='yt',
                                                 name=f'yt{t}')
                        nc.vector.tensor_copy(
                            box['yt'][:, NC * half:NC * (half + 1)],
                            box[half][:])
                        if half == 1:
                            nc.gpsimd.dma_start(y[ts_, :], box['yt'][:])
                    return fn

                return [(None, 1100, mk_half_a(0)), (None, 350, mk_half_b(0)),
                        (None, 1100, mk_half_a(1)),
                        (('wo', t), 350, mk_half_b(1))]

            # feeder fill order follows wavefront consumption order
            push(prep_item(0, 0, [0], 'scalar'))
            push(prep_item(0, 1, [0], 'scalar'))
            for t in range(4):
                push(v_item(t, 'scalar'))
            push(prep_item(0, 0, [1], 'scalar'))
            push(prep_item(0, 1, [1], 'scalar'))
            push(prep_item(1, 0, [0, 1], 'vector'))
            push(prep_item(1, 1, [0, 1], 'vector'))
            for t in range(4, 8):
                push(v_item(t, 'scalar'))
            push(prep_item(2, 0, [0, 1], 'vector'))
            push(prep_item(2, 1, [0, 1], 'vector'))
            push(prep_item(0, 0, [2, 3], 'vector'))
            push(prep_item(0, 1, [2, 3], 'vector'))
            for t in range(8, 12):
                push(v_item(t, 'scalar'))
            push(prep_item(3, 0, [0, 1], 'vector'))
            push(prep_item(3, 1, [0, 1], 'vector'))
            push(prep_item(1, 0, [2, 3], 'vector'))
            push(prep_item(1, 1, [2, 3], 'vector'))
            for t in range(12, 16):
                push(v_item(t, 'scalar'))
            push(prep_item(2, 0, [2, 3], 'vector'))
            push(prep_item(2, 1, [2, 3], 'vector'))
            push(prep_item(3, 0, [2, 3], 'vector'))
            push(prep_item(3, 1, [2, 3], 'vector'))

            # ---------- attention ----------
            def emit_att_chunk(g, c, drain_all=False):
                qt, kt_ = qk_tiles[g]
                hA, hB = 2 * g, 2 * g + 1
                cs = slice(NC * c, NC * (c + 1))
                pv = pvps.tile([128, NC], F32, tag='pv')
                den = denps.tile([128, NC], F32, tag='den')
                n_kt = 4 * (c + 1)
                p_tiles = {}

                def emit_sc(i):
                    drain(('v', i))
                    j = i - 4 * c
                    v0 = max(j, 0) * 128
                    w = NC - v0
                    qs = slice(NC * c + v0, NC * (c + 1))
                    ks = slice(128 * i, 128 * (i + 1))
                    sc = scps.tile([128, 2 * NC], F32, tag='sc')
                    nc.tensor.matmul(sc[:, v0:NC], kt_[0:64, ks],
                                     qt[0:64, qs], start=True, stop=True)
                    nc.tensor.matmul(sc[:, NC + v0:2 * NC], kt_[64:128, ks],
                                     qt[64:128, qs], start=True, stop=True)
                    p = pp.tile([128, 2 * NC], BF16, tag='p')
                    p_tiles[i] = p
                    sc_seg = bass.AP(tensor=sc.tensor, offset=sc.offset + v0,
                                     ap=[list(sc.ap[0]), [NC, 2], [1, w]])
                    p_seg = bass.AP(tensor=p.tensor, offset=p.offset + v0,
                                    ap=[list(p.ap[0]), [NC, 2], [1, w]])
                    nc.scalar.activation(out=p_seg, in_=sc_seg, func=EXP,
                                         scale=1.0 / np.sqrt(DK))
                    if j >= 0:       # triangle on the diag sub-block
                        dseg = bass.AP(
                            tensor=p.tensor, offset=p.offset + v0,
                            ap=[list(p.ap[0]), [NC, 2], [1, 128]])
                        nc.vector.tensor_tensor(
                            dseg, dseg,
                            tri_sb[:, None, :].to_broadcast([128, 2, 128]),
                            MUL)

                def emit_pv(i):
                    v0 = max(i - 4 * c, 0) * 128
                    sA = slice(v0, NC)
                    sB = slice(NC + v0, 2 * NC)
                    st_, sp_ = (i == 0), (i == n_kt - 1)
                    p = p_tiles[i]
                    nc.tensor.matmul(pv[0:64, sA], v_sb[:, i, hA, :],
                                     p[:, sA], start=st_, stop=sp_)
                    nc.tensor.matmul(pv[64:128, sA], v_sb[:, i, hB, :],
                                     p[:, sB], start=st_, stop=sp_)

                def emit_den2(i):
                    # c==0: later blocks are width-trimmed, so keep a single
                    # accumulator pair (rows 0/32) that block 0 fully covers
                    v0 = max(i - 4 * c, 0) * 128
                    sA = slice(v0, NC)
                    sB = slice(NC + v0, 2 * NC)
                    st_, sp_ = (i == 0), (i == n_kt - 1)
                    p = p_tiles.pop(i)
                    nc.tensor.matmul(den[0:1, sA], ones_sb[:], p[:, sA],
                                     start=st_, stop=sp_, tile_position=(0, 0))
                    nc.tensor.matmul(den[32:33, sA], ones_sb[:], p[:, sB],
                                     start=st_, stop=sp_, tile_position=(0, 32))

                def emit_den4(m):
                    # kt pair (2m, 2m+1): 4 ones-matmuls packed across PE
                    # column strips run concurrently -> one w-col pass per pair
                    st_, sp_ = (m == 0), (m == n_kt // 2 - 1)
                    for idx, i in enumerate((2 * m, 2 * m + 1)):
                        v0 = max(i - 4 * c, 0) * 128
                        sA = slice(v0, NC)
                        sB = slice(NC + v0, 2 * NC)
                        p = p_tiles.pop(i)
                        ro = 64 * idx
                        nc.tensor.matmul(den[ro:ro + 1, sA], ones_sb[:],
                                         p[:, sA], start=st_, stop=sp_,
                                         tile_position=(0, ro))
                        nc.tensor.matmul(den[ro + 32:ro + 33, sA], ones_sb[:],
                                         p[:, sB], start=st_, stop=sp_,
                                         tile_position=(0, ro + 32))

                for i in range(n_kt):
                    emit_sc(i)
                    # den work lags 2 extra blocks: its p inputs are long
                    # ready, so it streams stall-free between sc and pv
                    if c == 0:
                        if i >= 2:
                            emit_den2(i - 2)
                    elif i >= 4 and i % 2 == 0:
                        emit_den4((i - 4) // 2)
                    if drain_all and i == n_kt // 2:
                        while feeder:
                            run_next()
                    pull(1.25 * state['ns'] / max(state['kts'], 1))
                    state['kts'] -= 1
                    if i >= 1:
                        emit_pv(i - 1)
                emit_pv(n_kt - 1)
                if c == 0:
                    emit_den2(n_kt - 2)
                    emit_den2(n_kt - 1)
                else:
                    emit_den4(n_kt // 2 - 2)
                    emit_den4(n_kt // 2 - 1)

                # evac pv early (frees the single pv bank), normalize later
                nc.vector.tensor_copy(lhs_sb[:, g, cs], pv[:])

                stage = dent.tile([128, NC], F32, tag='dstage')
                hi = 33 if c == 0 else 97
                nc.vector.tensor_copy(stage[0:hi, :], den[0:hi, :])
                packed = dent.tile([128, 8], F32, tag='dpA')
                nc.sync.dma_start(packed[0:64, :], stage[0:1, :])
                nc.sync.dma_start(packed[64:128, :], stage[32:33, :])
                if c > 0:
                    packed2 = dent.tile([128, 8], F32, tag='dpB')
                    nc.gpsimd.dma_start(packed2[0:64, :], stage[64:65, :])
                    nc.gpsimd.dma_start(packed2[64:128, :], stage[96:97, :])
                    nc.vector.tensor_add(packed[:], packed[:], packed2[:])
                packr = dent.tile([128, 8], BF16, tag='dpackr')
                with nc.allow_low_precision(reason='softmax 1/sum in bf16'):
                    nc.vector.reciprocal(packr[:], packed[:])
                flat = dent.tile([128, 2 * NC], BF16, tag='dflat')
                nc.gpsimd.dma_start(flat[0:1, :], packr[:])
                rcf = rct.tile([128, 2 * NC], BF16, tag='rcf')
                nc.gpsimd.partition_broadcast(rcf[:], flat[0:1, :])
                nc.vector.tensor_tensor(lhs_sb[0:64, g, cs],
                                        lhs_sb[0:64, g, cs],
                                        rcf[0:64, 0:NC], MUL)
                nc.vector.tensor_tensor(lhs_sb[64:128, g, cs],
                                        lhs_sb[64:128, g, cs],
                                        rcf[64:128, NC:2 * NC], MUL)

            # ------------- anti-diagonal wavefront -------------
            done_cnt = [0] * N_CHUNKS
            last_diag = N_CHUNKS + NG - 2
            for k in range(N_CHUNKS + NG - 1):
                for g in range(max(0, k - N_CHUNKS + 1), min(NG - 1, k) + 1):
                    c = k - g
                    drain(('pc', g, 0, c))
                    for cc in range(c + 1):
                        drain(('pc', g, 1, cc))
                    emit_att_chunk(g, c, drain_all=(k == last_diag))
                    done_cnt[c] += 1
                    if done_cnt[c] == NG:
                        if c < N_CHUNKS - 1:
                            hi = 4 * (c + 1) - (2 if c == N_CHUNKS - 2 else 0)
                            for t in range(4 * c, hi):
                                push(wo_item(t))
                        else:
                            while feeder:
                                run_next()
                            # t10/t11 (chunk 2, already normalized) lead the
                            # final waves as chain-latency filler
                            units = [(t, half)
                                     for t in range(4 * c - 2, 4 * (c + 1))
                                     for half in range(2)]
                            yts = {}
                            pools = [projps, projps, pvps, denps]
                            tags = ['proj', 'proj', 'pv', 'den']
                            ydisp = [nc.sync, nc.gpsimd]
                            for wi, w0 in enumerate(range(0, len(units), 4)):
                                wave = units[w0:w0 + 4]
                                accs = {}
                                for ui, (t, half) in enumerate(wave):
                                    accs[(t, half)] = pools[ui].tile(
                                        [128, NC], F32, tag=tags[ui],
                                        name=f'pwof{t}_{half}')
                                for gg in range(NG):
                                    for (t, half) in wave:
                                        ts_ = slice(128 * t, 128 * (t + 1))
                                        nc.tensor.matmul(
                                            accs[(t, half)][:],
                                            lhs_sb[:, gg, ts_],
                                            wo_sb[:, gg,
                                                  NC * half:NC * (half + 1)],
                                            start=(gg == 0),
                                            stop=(gg == NG - 1))
                                for (t, half) in wave:
                                    ts_ = slice(128 * t, 128 * (t + 1))
                                    if t not in yts:
                                        yts[t] = ytp.tile([128, D], BF16,
                                                          tag='yt',
                                                          name=f'ytf{t}')
                                    nc.scalar.copy(
                                        yts[t][:, NC * half:NC * (half + 1)],
                                        accs[(t, half)][:])
                                    if half == 1:
                                        ydisp[t % 2].dma_start(y[ts_, :],
                                                               yts[t][:])
    nc.compile()
    return nc


def _host_inputs(x, Wq, Wk, Wv, Wo, token_positions):
    """Per-core input maps (host-side sharding / layout / dtype prep only)."""
    import ml_dtypes
    bf16 = ml_dtypes.bfloat16
    perm = np.empty(DK, np.int64)
    perm[0:32] = np.arange(0, DK, 2)
    perm[32:64] = np.arange(1, DK, 2)

    inv_freq = 1.0 / (THETA ** (np.arange(0, DK, 2, dtype=np.float64) / DK))  # [32]
    ang = token_positions.astype(np.float64)[None, :] * inv_freq[:, None]     # [32, S]
    cos32 = np.cos(ang).astype(np.float32)
    sin32 = np.sin(ang).astype(np.float32)
    cos128 = np.tile(cos32, (4, 1)).astype(bf16)
    sin128 = np.concatenate([-sin32, sin32, -sin32, sin32], axis=0).astype(bf16)

    tri = (np.arange(128)[None, :] >= np.arange(128)[:, None]).astype(bf16)

    in_maps = []
    for core in range(8):
        b = core // 2
        h0 = (core % 2) * NHC
        cols = slice(h0 * DK, (h0 + NHC) * DK)
        wq_s = Wq[cols, :].reshape(NHC, DK, D)[:, perm, :].reshape(HD, D)
        wk_s = Wk[cols, :].reshape(NHC, DK, D)[:, perm, :].reshape(HD, D)
        in_maps.append({
            'xT': np.ascontiguousarray(x[b].T.astype(bf16)),
            'wqT': np.ascontiguousarray(wq_s.T.astype(bf16)),
            'wkT': np.ascontiguousarray(wk_s.T.astype(bf16)),
            'wvT': np.ascontiguousarray(Wv[cols, :].T.astype(bf16)),
            'woT': np.ascontiguousarray(Wo[:, cols].T.astype(bf16)),
            'cosd': cos128, 'sind': sin128, 'maskd': tri,
        })
    return in_maps


def kernel(x, Wq, Wk, Wv, Wo, token_positions, _results_hook=None):
    if 'nc' not in _CACHED:
        _CACHED['nc'] = _build()
    nc = _CACHED['nc']
    in_maps = _host_inputs(np.asarray(x), np.asarray(Wq), np.asarray(Wk),
                           np.asarray(Wv), np.asarray(Wo),
                           np.asarray(token_positions))
    res = run_bass_kernel_spmd(nc, in_maps, list(range(8)),
                               **(_results_hook or {}))
    if _results_hook is not None:
        _CACHED['last'] = res
    out = np.empty((B, S, D), np.float32)
    for b in range(B):
        out[b] = (res.results[2 * b]['y'].astype(np.float32)
                  + res.results[2 * b + 1]['y'].astype(np.float32))
    return out
